# revision 2
# baseline (speedup 1.0000x reference)
# ChebConv (K=4) + BatchNorm + LeakyReLU, distributed over 8 TRN2 NeuronCores.
#
# Sharding: nodes split into M=8 contiguous shards (12500 nodes/core). Edges are
# partitioned by destination core; inside a core they are grouped by
# (dst window of 128 nodes, src chunk) where the src chunks are 4 window-aligned
# row ranges of each shard (so chunk tables stay addressable by int16 gather
# indices: 8*3200 < 32767).
#
# Math: with dinv = deg^-1/2 (computed on host from edge_idx, shipped as a
# [P, W] per-core input), prop(v)[c] = -dinv[c] * sum_{e: col=c} (dinv*v)[row_e]
# so each round gathers from a pre-scaled table u_k = dinv ⊙ T_k (AllGathered in
# 4 chunks per round to overlap communication with compute) and post-scales by
# -dinv (or -2*dinv for the Chebyshev recurrence) per destination window.
# Segment sums are computed as S^T @ U on the PE where S[e, d] = 1{col_e == d}
# is built on DVE via is_equal against an iota row. The output accumulates in a
# transposed [f_out, nodes] layout so BatchNorm statistics (reduce over nodes)
# and the affine + LeakyReLU epilogue are cheap per-partition ops; the host
# transposes shards back when assembling the full output. The ChebConv bias b
# cancels exactly through BatchNorm, so it is ignored.

import numpy as np

from concourse import bass, bacc, mybir
import concourse.tile as tile
from concourse.masks import make_identity
from concourse.library_config import mlp as mlp_lib

P = 128
F = 128
FP32 = mybir.dt.float32
I16 = mybir.dt.int16
I32 = mybir.dt.int32
AOp = mybir.AluOpType
AF = mybir.ActivationFunctionType
AX = mybir.AxisListType
BN_EPS = 1e-5
LEAKY = 0.01
MAXG = 1024  # dma_gather breaks above 1024 idxs per instruction


def _cdiv(a, b):
    return -(-a // b)


def plan(edge_idx, N, M, nch=4):
    """Host-side layout prep: edge partitioning/sorting + packed index arrays."""
    row = np.asarray(edge_idx[0], dtype=np.int64)
    col = np.asarray(edge_idx[1], dtype=np.int64)
    shard = N // M
    assert shard * M == N
    W = _cdiv(shard, P)
    win_rows = [min(P, shard - w * P) for w in range(W)]

    base, rem = W // nch, W % nch
    ch_nw = [base + (1 if c < rem else 0) for c in range(nch)]
    ch_w0 = np.cumsum([0] + ch_nw)[:-1].tolist()
    ch_r0 = [min(w0 * P, shard) for w0 in ch_w0]
    ch_rows = []
    for c in range(nch):
        r1 = min((ch_w0[c] + ch_nw[c]) * P, shard)
        ch_rows.append(r1 - ch_r0[c])
    assert all(M * r <= 32767 for r in ch_rows), (M, ch_rows)

    dst_core = col // shard
    dloc = col % shard
    win = dloc // P
    col_in_win = dloc % P
    src_core = row // shard
    sloc = row % shard
    ch_bounds = np.array(ch_r0 + [shard], dtype=np.int64)
    src_ch = np.searchsorted(ch_bounds, sloc, side="right") - 1
    idx16 = src_core * np.asarray(ch_rows, dtype=np.int64)[src_ch] + (
        sloc - ch_bounds[src_ch]
    )

    # main groups keyed (dst_core, src_ch, win) — bank(chunk)-major processing
    gkey = (dst_core * nch + src_ch) * W + win
    counts = np.bincount(gkey, minlength=M * nch * W).reshape(M, nch, W)
    caps = np.zeros((nch, W), dtype=np.int64)
    for c in range(nch):
        for w in range(W):
            mx = counts[:, c, w].max()
            caps[c][w] = _cdiv(mx, P) * P if mx > 0 else 0

    # padded offsets, (c, w) order
    off_pad = np.zeros((nch, W), dtype=np.int64)
    t = 0
    for c in range(nch):
        for w in range(W):
            off_pad[c][w] = t
            t += caps[c][w]
    tot_pad = t

    order = np.lexsort((gkey, dst_core))
    idx16_arrs, colloc_arrs = [], []
    for m in range(M):
        sel = order[dst_core[order] == m]
        k = gkey[sel] % (nch * W)  # (c, w) flat index
        # secondary sort by table index within each group: the gather
        # descriptors then sweep the table in ascending address order,
        # which improves HBM row locality for the random reads.
        ks = np.lexsort((idx16[sel], k))
        sel = sel[ks]
        k = k[ks]
        # rank within group
        grp_start = np.searchsorted(k, np.arange(nch * W))
        j = np.arange(sel.size) - grp_start[k]
        pos = off_pad.reshape(-1)[k] + j
        idx_flat = np.zeros(tot_pad, dtype=np.int16)  # pad idx 0 = valid row
        cl_flat = np.full(tot_pad, -1.0, dtype=np.float32)
        idx_flat[pos] = idx16[sel].astype(np.int16)
        cl_flat[pos] = col_in_win[sel].astype(np.float32)
        ia = np.zeros((16, tot_pad // 16), dtype=np.int16)
        ia[pos % 16, pos // 16] = idx_flat[pos]
        idx16_arrs.append(np.tile(ia, (8, 1)))
        ca = np.full((P, tot_pad // P), -1.0, dtype=np.float32)
        ca[pos % P, pos // P] = cl_flat[pos]
        colloc_arrs.append(ca)

    # host-side degree -> dinv per core as a [P, W] window-layout array
    deg = np.bincount(row, minlength=N).astype(np.float64)
    dinv = np.where(deg > 0, 1.0 / np.sqrt(np.maximum(deg, 1e-12)), 0.0)
    dinv_arrs = []
    for m in range(M):
        da = np.zeros((P, W), dtype=np.float32)
        sh = dinv[m * shard:(m + 1) * shard]
        for w in range(W):
            r0 = w * P
            rw = min(P, shard - r0)
            da[:rw, w] = sh[r0: r0 + rw]
        dinv_arrs.append(da)

    return dict(
        N=N, M=M, shard=shard, W=W, win_rows=win_rows, nch=nch,
        ch_nw=ch_nw, ch_w0=ch_w0, ch_r0=ch_r0, ch_rows=ch_rows,
        caps=caps, off_pad=off_pad, tot_pad=tot_pad,
        idx16_arrs=idx16_arrs, colloc_arrs=colloc_arrs, dinv_arrs=dinv_arrs,
    )


def which_chunk(ch_w0, ch_nw, w):
    for c in range(len(ch_w0)):
        if ch_w0[c] <= w < ch_w0[c] + ch_nw[c]:
            return c
    raise AssertionError


def ch_w0_last(ch_w0, ch_nw, w):
    c = which_chunk(ch_w0, ch_nw, w)
    return ch_w0[c] + ch_nw[c] - 1


def build(nc, cfg, K, no_cc=False, reps=1):
    M, shard, W, nch = cfg["M"], cfg["shard"], cfg["W"], cfg["nch"]
    win_rows, caps, off_pad = cfg["win_rows"], cfg["caps"], cfg["off_pad"]
    ch_nw, ch_w0, ch_r0, ch_rows = (
        cfg["ch_nw"], cfg["ch_w0"], cfg["ch_r0"], cfg["ch_rows"],
    )
    N = cfg["N"]
    rg = [list(range(M))]
    shared_as = "Shared" if M > 4 else "Local"
    capmax = int(max(caps.max(), 1))

    x_d = nc.dram_tensor("x_sh", [shard, F], FP32, kind="ExternalInput").ap()
    w_d = nc.dram_tensor("w_all", [K, F, F], FP32, kind="ExternalInput").ap()
    gam_d = nc.dram_tensor("gamma", [F, 1], FP32, kind="ExternalInput").ap()
    bet_d = nc.dram_tensor("beta", [F, 1], FP32, kind="ExternalInput").ap()
    idx_d = nc.dram_tensor(
        "idx16", [P, cfg["tot_pad"] // 16], I16, kind="ExternalInput"
    ).ap()
    cl_d = nc.dram_tensor(
        "colloc", [P, cfg["tot_pad"] // P], FP32, kind="ExternalInput"
    ).ap()
    dinv_d = nc.dram_tensor("dinv_in", [P, W], FP32, kind="ExternalInput").ap()
    out_d = nc.dram_tensor("out_t", [P, shard], FP32, kind="ExternalOutput").ap()

    with tile.TileContext(nc) as tc:
        with (
            tc.tile_pool(name="persist", bufs=1) as pp,
            tc.tile_pool(name="stage", bufs=5) as sp,
            tc.tile_pool(name="sbuild", bufs=6) as sbp,
            tc.tile_pool(name="vec", bufs=4) as vp,
            tc.tile_pool(name="roll", bufs=2) as rp,
            tc.tile_pool(name="ps_g", bufs=4, space="PSUM") as pg,
            tc.tile_pool(name="ps_sm", bufs=2, space="PSUM") as psm,
            tc.tile_pool(name="ps_o", bufs=2, space="PSUM") as po,
            tc.tile_pool(name="dram", bufs=1, space="DRAM") as dp,
        ):
            # ---- persistent SBUF
            A = pp.tile([P, W * F], FP32, name="Abuf")
            B = pp.tile([P, W * F], FP32, name="Bbuf")
            outT = pp.tile([P, shard], FP32, name="outT")
            ident = pp.tile([P, P], FP32, name="ident")
            iota_i = pp.tile([P, P], I32, name="iota_i")
            iota_f = pp.tile([P, P], FP32, name="iota_f")
            W_sb = pp.tile([P, K * F], FP32, name="W_sb")
            gam = pp.tile([P, 1], FP32, name="gam")
            bet = pp.tile([P, 1], FP32, name="bet")
            dinv = pp.tile([P, W], FP32, name="dinv")
            nd1 = pp.tile([P, W], FP32, name="nd1")
            nd2 = pp.tile([P, W], FP32, name="nd2")
            eps_t = pp.tile([P, 1], FP32, name="eps_t")

            make_identity(nc, ident[:])
            nc.gpsimd.iota(iota_i[:], pattern=[[1, P]], base=0, channel_multiplier=0)
            nc.gpsimd.load_library(mlp_lib)
            nc.vector.tensor_copy(iota_f[:], iota_i[:])
            nc.vector.memset(eps_t[:], BN_EPS)
            nc.vector.memset(A[:], 0.0)
            nc.vector.memset(B[:], 0.0)
            for k in range(K):
                nc.sync.dma_start(W_sb[:, k * F:(k + 1) * F], w_d[k])
            nc.sync.dma_start(gam[:], gam_d[:])
            nc.sync.dma_start(bet[:], bet_d[:])
            nc.sync.dma_start(dinv[:], dinv_d[:])
            nc.scalar.mul(nd1[:], dinv[:], -1.0)
            nc.scalar.mul(nd2[:], dinv[:], -2.0)

            cap_regs = {}

            def cap_reg(cap):
                if cap not in cap_regs:
                    cap_regs[cap] = nc.gpsimd.to_reg(cap)
                return cap_regs[cap]

            def wslice(buf, w):
                return buf[:, w * F:(w + 1) * F]

            def emit_rep(rep):
                sfx = f"_r{rep}"
                u_in = [
                    dp.tile([ch_rows[c], F], FP32, name=f"u_in{c}{sfx}")
                    for c in range(nch)
                ]
                u_out = [
                    [
                        dp.tile(
                            [M * ch_rows[c], F], FP32,
                            name=f"u_out{c}_{kr}{sfx}", addr_space=shared_as,
                        )
                        for kr in range(K - 1)
                    ]
                    for c in range(nch)
                ]
                bn_in = dp.tile([P, 2], FP32, name=f"bn_in{sfx}")
                bn_out = dp.tile([P, 2], FP32, name=f"bn_out{sfx}",
                                 addr_space=shared_as)

                # x shard -> A  (A[p, w*F + f] = x[w*128 + p, f])
                full = shard // P
                if full:
                    nc.sync.dma_start(
                        A[:].rearrange("p (w f) -> p w f", f=F)[:, :full, :],
                        x_d[: full * P, :].rearrange("(w p) f -> p w f", p=P),
                    )
                rem = shard - full * P
                if rem:
                    nc.sync.dma_start(
                        A[:rem, full * F:(full + 1) * F], x_d[full * P:, :]
                    )

                # ---- W_k projection: out^T += W_k^T @ T_k^T per 4-window chunk
                def wk_chain(src_buf, k, w, troll_box):
                    q, pos = w // 4, w % 4
                    if pos == 0:
                        troll_box[0] = rp.tile([P, 4 * P], FP32, name="troll")
                    troll = troll_box[0]
                    ps_t = psm.tile([P, P], FP32, name="ps_small", tag="ps_small")
                    nc.tensor.transpose(ps_t[:], wslice(src_buf, w), ident[:])
                    rw = win_rows[w]
                    nc.scalar.copy(troll[:, pos * P: pos * P + rw], ps_t[:, :rw])
                    if w == min(4 * q + 3, W - 1):
                        node0 = q * 4 * P
                        ncols = min(4 * P, shard - node0)
                        ps_o = po.tile([P, 4 * P], FP32, name="ps_o")
                        nc.tensor.matmul(
                            ps_o[:, :ncols],
                            lhsT=W_sb[:, k * F:(k + 1) * F],
                            rhs=troll[:, :ncols],
                            start=True, stop=True,
                        )
                        sl = outT[:, node0: node0 + ncols]
                        if k == 0:
                            nc.vector.tensor_copy(sl, ps_o[:, :ncols])
                        else:
                            nc.vector.tensor_tensor(
                                sl, sl, ps_o[:, :ncols], op=AOp.add
                            )

                def u_write(src_buf, w):
                    c2 = which_chunk(ch_w0, ch_nw, w)
                    us = sp.tile([P, F], FP32, name="us")
                    nc.scalar.mul(us[:], wslice(src_buf, w), dinv[:, w: w + 1])
                    r0 = w * P - ch_r0[c2]
                    rw = win_rows[w]
                    nc.sync.dma_start(u_in[c2][r0: r0 + rw, :], us[:rw, :])

                def fire_ag(c, kround):
                    if no_cc:
                        return
                    nc.gpsimd.collective_compute(
                        "AllGather", AOp.bypass, replica_groups=rg,
                        ins=[u_in[c].opt()], outs=[u_out[c][kround].opt()],
                    )

                # ========== round 0: u0 writes + W0 projection =============
                troll_box = [None]
                for w in range(W):
                    wk_chain(A, 0, w, troll_box)
                for c in range(nch):
                    for w in range(ch_w0[c], ch_w0[c] + ch_nw[c]):
                        u_write(A, w)
                    fire_ag(c, 0)

                # ========== rounds 1..K-1 ====================================
                for k in range(1, K):
                    dst = B if k % 2 == 1 else A
                    nd = nd1 if k == 1 else nd2
                    troll_box = [None]
                    for c in range(nch):
                        tab = u_out[c][k - 1]
                        for w in range(W):
                            cap = int(caps[c][w])
                            first = all(caps[cc][w] == 0 for cc in range(c))
                            if cap > 0:
                                g = cap // P
                                o16 = int(off_pad[c][w]) // 16
                                ot = int(off_pad[c][w]) // P
                                it = sp.tile([P, capmax // 16], I16, name="it")
                                clt = sp.tile([P, capmax // P], FP32, name="clt")
                                nc.sync.dma_start(
                                    it[:, : cap // 16],
                                    idx_d[:, o16: o16 + cap // 16],
                                )
                                nc.sync.dma_start(clt[:, :g], cl_d[:, ot: ot + g])
                                U = sp.tile([P, capmax], FP32, name="Ug")
                                Uv = U[:].rearrange("p (g f) -> p g f", f=F)
                                for goff in range(0, cap, MAXG):
                                    sub = min(MAXG, cap - goff)
                                    nc.gpsimd.dma_gather(
                                        out_ap=Uv[:, goff // P:(goff + sub) // P, :],
                                        in_ap=tab[:],
                                        idxs_ap=it[:, goff // 16:(goff + sub) // 16],
                                        num_idxs=sub,
                                        num_idxs_reg=cap_reg(sub),
                                        elem_size=F,
                                    )
                                ps_g = pg.tile([P, P], FP32, name="ps_g")
                                for t in range(g):
                                    S = sbp.tile([P, P], FP32, name="Sb")
                                    nc.vector.tensor_scalar(
                                        out=S[:], in0=iota_f[:],
                                        scalar1=clt[:, t: t + 1], scalar2=None,
                                        op0=AOp.is_equal,
                                    )
                                    nc.tensor.matmul(
                                        ps_g[:], lhsT=S[:], rhs=Uv[:, t, :],
                                        start=(t == 0), stop=(t == g - 1),
                                    )
                                dw = wslice(dst, w)
                                if first and k == 1:
                                    nc.vector.tensor_scalar_mul(
                                        out=dw, in0=ps_g[:],
                                        scalar1=nd[:, w: w + 1],
                                    )
                                else:
                                    nc.vector.scalar_tensor_tensor(
                                        out=dw, in0=ps_g[:],
                                        scalar=nd[:, w: w + 1],
                                        in1=dw, op0=AOp.mult,
                                        op1=(AOp.subtract if first else AOp.add),
                                    )
                            elif first and c == nch - 1 and all(
                                caps[cc][w] == 0 for cc in range(nch)
                            ):
                                # isolated window: T_k = (k==1 ? 0 : -T_{k-2})
                                dw = wslice(dst, w)
                                if k == 1:
                                    nc.vector.memset(dw, 0.0)
                                else:
                                    nc.vector.tensor_scalar_mul(
                                        out=dw, in0=dw, scalar1=-1.0
                                    )
                            if c == nch - 1:
                                if k < K - 1:
                                    u_write(dst, w)
                                wk_chain(dst, k, w, troll_box)
                                if k < K - 1 and w == ch_w0_last(ch_w0, ch_nw, w):
                                    fire_ag(which_chunk(ch_w0, ch_nw, w), k)

                # ========== BatchNorm + LeakyReLU ============================
                s1 = vp.tile([P, 1], FP32, name="d1")
                nc.vector.reduce_sum(out=s1[:], in_=outT[:, :shard], axis=AX.X)
                s2 = pp.tile([P, 1], FP32, name="s2acc")
                nc.vector.memset(s2[:], 0.0)
                CH = 4 * P
                for n0 in range(0, shard, CH):
                    n1 = min(n0 + CH, shard)
                    sqs = rp.tile([P, CH], FP32, name="sqs")
                    s2p = vp.tile([P, 1], FP32, name="mk")
                    nc.scalar.activation(
                        sqs[:, : n1 - n0], outT[:, n0:n1], AF.Square,
                        accum_out=s2p[:],
                    )
                    nc.vector.tensor_tensor(s2[:], s2[:], s2p[:], op=AOp.add)
                bn_sb = pp.tile([P, 2], FP32, name="bn_sb")
                nc.vector.tensor_copy(bn_sb[:, 0:1], s1[:])
                nc.vector.tensor_copy(bn_sb[:, 1:2], s2[:])
                nc.sync.dma_start(bn_in[:], bn_sb[:])
                if not no_cc:
                    nc.gpsimd.collective_compute(
                        "AllReduce", AOp.add, replica_groups=rg,
                        ins=[bn_in.opt()], outs=[bn_out.opt()],
                    )
                bnr = pp.tile([P, 2], FP32, name="bnr")
                nc.sync.dma_start(bnr[:], bn_out[:])
                mean = vp.tile([P, 1], FP32, name="d1")
                msq = vp.tile([P, 1], FP32, name="mk")
                nc.scalar.mul(mean[:], bnr[:, 0:1], 1.0 / N)
                nc.scalar.mul(msq[:], bnr[:, 1:2], 1.0 / N)
                m2 = vp.tile([P, 1], FP32, name="sq")
                var = vp.tile([P, 1], FP32, name="rc")
                nc.vector.tensor_tensor(m2[:], mean[:], mean[:], op=AOp.mult)
                nc.vector.tensor_tensor(var[:], msq[:], m2[:], op=AOp.subtract)
                stdv = pp.tile([P, 1], FP32, name="stdv")
                rstd = pp.tile([P, 1], FP32, name="rstd")
                nc.scalar.activation(stdv[:], var[:], AF.Sqrt, bias=eps_t[:])
                nc.vector.reciprocal(rstd[:], stdv[:])
                Aaff = pp.tile([P, 1], FP32, name="Aaff")
                Baff = pp.tile([P, 1], FP32, name="Baff")
                mA = vp.tile([P, 1], FP32, name="d1")
                nc.vector.tensor_tensor(Aaff[:], gam[:], rstd[:], op=AOp.mult)
                nc.vector.tensor_tensor(mA[:], mean[:], Aaff[:], op=AOp.mult)
                nc.vector.tensor_tensor(Baff[:], bet[:], mA[:], op=AOp.subtract)
                for n0 in range(0, shard, CH):
                    n1 = min(n0 + CH, shard)
                    ts = rp.tile([P, CH], FP32, name="sqs")
                    nc.scalar.activation(
                        ts[:, : n1 - n0], outT[:, n0:n1], AF.Identity,
                        bias=Baff[:], scale=Aaff[:],
                    )
                    nc.vector.scalar_tensor_tensor(
                        out=outT[:, n0:n1], in0=ts[:, : n1 - n0], scalar=LEAKY,
                        in1=ts[:, : n1 - n0], op0=AOp.mult, op1=AOp.max,
                    )
                nc.sync.dma_start(out_d[:], outT[:, :shard])

            for rep in range(reps):
                emit_rep(rep)
    return nc


def make_in_maps(cfg, x, W_, gamma, beta):
    M, shard = cfg["M"], cfg["shard"]
    x = np.asarray(x, dtype=np.float32)
    maps = []
    for m in range(M):
        maps.append(
            {
                "x_sh": np.ascontiguousarray(x[m * shard:(m + 1) * shard]),
                "w_all": np.asarray(W_, dtype=np.float32),
                "gamma": np.asarray(gamma, dtype=np.float32).reshape(F, 1),
                "beta": np.asarray(beta, dtype=np.float32).reshape(F, 1),
                "idx16": cfg["idx16_arrs"][m],
                "colloc": cfg["colloc_arrs"][m],
                "dinv_in": cfg["dinv_arrs"][m],
            }
        )
    return maps


def assemble(cfg, results):
    M, shard = cfg["M"], cfg["shard"]
    out = np.empty((M * shard, F), dtype=np.float32)
    for m in range(M):
        out[m * shard:(m + 1) * shard] = results[m]["out_t"].T
    return out


def kernel(x, edge_idx, W, b, gamma, beta):
    from concourse.bass_utils import run_bass_kernel_spmd

    M = 8
    N = x.shape[0]
    K = W.shape[0]
    cfg = plan(np.asarray(edge_idx), N, M, nch=4)
    nc = bacc.Bacc("TRN2", num_devices=M)
    build(nc, cfg, K)
    nc.compile()
    in_maps = make_in_maps(cfg, x, W, gamma, beta)
    res = run_bass_kernel_spmd(nc, in_maps, core_ids=list(range(M)))
    return assemble(cfg, res.results)

